# revision 8
# baseline (speedup 1.0000x reference)
"""Trainium2 Bass kernel for nn_CONV_A_64115271795341.

The module (im2col mean-centered conv + linear on window means) folds into
a single 3x3 edge-padded conv with host-folded effective weights
  W_eff[c,k,d] = weight[c,k,d] + (w_lin[d,c] - sum_k weight[c,k,d]) / 9.
Matmuls execute serially on the PE queue here, so wall time ~ #MM x N;
this kernel packs 3 kernel taps into every matmul (99 total vs 192 in the
naive pair/single decomposition):

  - SBUF xp[128, NP] fp16: partitions 0-63 = padded image (row-major,
    WP=130), partitions 64-127 = same shifted one row (host-prepped), so
    K=128 contracts two vertically adjacent taps at once.
  - For output span h0..h0+3 (N=512) and kernel col j, ONE matmul with
    rhs base row h0+1 and lhsT[128, 128]:
      cols 0-63  (alpha): rows 0-63 = W(1,j), rows 64-127 = W(2,j)
        -> psum[0:64]  += taps (1,j)+(2,j), aligned with the span.
      cols 64-127 (beta): rows 0-63 = W(0,j), rows 64-127 = 0
        -> psum[64:128] += tap (0,j) partials leading by one output row.
    j=0,1,2 accumulate -> 3 matmuls cover all 9 taps; output row 0's
    beta piece comes from 3 head matmuls (N=128) on x row 0 at rep start.
  - psum groups of SPG=2 spans (2 banks) x 4 pool buffers = all 8 banks,
    3 groups of slack between matmuls and the drain chain.
  - beta partials staged into sbB[64, W + H*W] at a +1-row offset (one
    partition-crossing ACT copy per group), laid out so sbB[Q*W] holds
    output row Q's partial: each DVE add reads a ZERO-shift window and
    fires immediately after its own group's stage.
  - out[q] = psTop[q] + sbB[q]; outt fp16, one output DMA.
  - 8 images data-parallel across 8 cores; weights replicated.
"""

import numpy as np

C, H, W, D, B = 64, 128, 128, 64, 8
KS = 3
WP = W + 2            # 130
HP = H + 2
NP = WP * HP          # 16900
TILE_ROWS = 4
TN = TILE_ROWS * W    # 512
NSPANS = H // TILE_ROWS   # 32
SPG = 2                   # spans per psum group
NG = NSPANS // SPG        # 16 groups
G = SPG * TN              # 1024 cols per group
SHIFT = 2 * W             # 256: beta partials lag 2 output rows

_CACHE = {}


def _build(repeat=1, in_chunks=4, out_chunks=1, psum_bufs=4, xp_bufs=2,
           out_bufs=2, stg_dtype="float32", dup="host", dup_chunks=8,
           spg=SPG, head="device", skip_in=False, skip_out=False,
           skip_drain=False, skip_mm=False):
    NGl = NSPANS // spg
    Gl = spg * TN
    import concourse.bass as bass  # noqa: F401
    import concourse.mybir as mybir
    import concourse.tile as tile
    from concourse import bacc

    dt = mybir.dt
    sdt = getattr(dt, stg_dtype)
    nc = bacc.Bacc("TRN2", target_bir_lowering=False, debug=False, num_devices=8)

    x_d = nc.dram_tensor("x", [128, NP], dt.float16, kind="ExternalInput")
    w_d = nc.dram_tensor("w", [128, KS * 128], dt.float16, kind="ExternalInput")
    out_d = nc.dram_tensor("out", [D, H * W], dt.float16, kind="ExternalOutput")

    with tile.TileContext(nc) as tc:
        with tc.tile_pool(name="io", bufs=xp_bufs) as io_pool, \
             tc.tile_pool(name="wp", bufs=2) as w_pool, \
             tc.tile_pool(name="outp", bufs=out_bufs) as out_pool, \
             tc.tile_pool(name="stg", bufs=1) as stg_pool, \
             tc.tile_pool(name="ps", bufs=psum_bufs, space="PSUM") as ps_pool:

            for _rep in range(repeat):
                w_sb = w_pool.tile([128, KS * 128], dt.float16,
                                   name="w_sb", tag="w_sb")
                nc.sync.dma_start(w_sb[:, :], w_d.ap()[:, :])

                xp = io_pool.tile([128, NP], dt.float16, name="xp", tag="xp")
                bnd = [NP * g // in_chunks for g in range(in_chunks + 1)]
                if not skip_in:
                    for g in range(in_chunks):
                        a, b = bnd[g], bnd[g + 1]
                        nc.sync.dma_start(xp[:, a:b], x_d.ap()[:, a:b])
                else:
                    nc.sync.dma_start(xp[:, 0:NP], x_d.ap()[:, 0:NP]) if False else                     nc.sync.dma_start(xp[:, 0:64], x_d.ap()[:, 0:64])

                xv = xp.rearrange("p (r c) -> p r c", c=WP)
                outt = sbB = None
                if not skip_drain:
                    outt = out_pool.tile([D, H * W], dt.float16,
                                         name="outt", tag="outt")
                    sbB = stg_pool.tile([64, H * W + W], sdt,
                                        name="sbB", tag="sbB")

                # head piece: beta (tap-row-0) partials for output row 0,
                # computed up front from x row 0 (available with chunk 0)
                if head == "host" and not skip_drain:
                    nc.vector.memset(sbB[:, 0:W], 0.0)
                if not (skip_drain or skip_mm or head == "host"):
                    psQ = ps_pool.tile([128, Gl], mybir.dt.float32,
                                       name="psP", tag="psP")
                    for j in range(KS):
                        nc.tensor.matmul(
                            psQ[0:64, 0:W],
                            lhsT=w_sb[0:64, 128 * j + 64:128 * j + 128],
                            rhs=xv[0:64, 0:1, j:j + W],
                            start=(j == 0), stop=(j == KS - 1),
                        )
                    nc.scalar.copy(sbB[:, 0:W], psQ[0:64, 0:W])

                ps_list = []
                for g in range(NGl):
                    if skip_mm:
                        break
                    psP = ps_pool.tile([128, Gl], mybir.dt.float32,
                                       name="psP", tag="psP")
                    ps_list.append(psP)
                    for s in range(spg if not skip_mm else 0):
                        h0 = TILE_ROWS * (spg * g + s)
                        for j in range(KS):
                            nc.tensor.matmul(
                                psP[:, TN * s:TN * (s + 1)],
                                lhsT=w_sb[:, 128 * j:128 * (j + 1)],
                                rhs=xv[:, h0 + 1:h0 + 1 + TILE_ROWS, j:j + W],
                                start=(j == 0), stop=(j == KS - 1),
                            )
                    # stage this group's beta partials contiguously
                    if skip_drain:
                        continue
                    nc.scalar.copy(sbB[:, W + Gl * g:W + Gl * (g + 1)],
                                   psP[64:128, :])
                    nc.vector.tensor_add(
                        outt[:, Gl * g:Gl * (g + 1)],
                        psP[0:64, :],
                        sbB[:, Gl * g:Gl * (g + 1)])

                obnd = [H * W * g // out_chunks for g in range(out_chunks + 1)]
                osrc = xp[0:64, 0:H * W] if skip_drain else outt
                if not skip_out:
                    for g in range(out_chunks):
                        a, b = obnd[g], obnd[g + 1]
                        nc.scalar.dma_start(out_d.ap()[:, a:b], osrc[:, a:b])

    nc.compile()
    return nc


def _prep_inputs(x, weight, w_lin):
    w = np.asarray(weight).astype(np.float64)
    weff = w + (np.asarray(w_lin).astype(np.float64).T[:, None, :]
                - w.sum(axis=1, keepdims=True)) / 9.0
    weff = weff.astype(np.float32)                      # [C, 9, D]
    w_sb = np.zeros((128, KS * 128), np.float16)
    for j in range(KS):
        w_sb[0:C, 128 * j:128 * j + 64] = weff[:, 1 * KS + j, :]
        w_sb[C:128, 128 * j:128 * j + 64] = weff[:, 2 * KS + j, :]
        w_sb[0:C, 128 * j + 64:128 * j + 128] = weff[:, 0 * KS + j, :]

    xpad = np.pad(np.asarray(x), ((0, 0), (0, 0), (1, 1), (1, 1)), mode="edge")
    xpad = xpad.reshape(B, C, NP).astype(np.float16)
    xh = np.zeros((B, 128, NP), np.float16)
    xh[:, 0:C, :] = xpad
    xh[:, C:128, 0:NP - WP] = xpad[:, :, WP:]
    return xh, w_sb


def kernel(x, weight, w_lin):
    from concourse.bass_utils import run_bass_kernel_spmd

    if "nc" not in _CACHE:
        _CACHE["nc"] = _build()
    nc = _CACHE["nc"]

    xh, w_sb = _prep_inputs(x, weight, w_lin)
    in_maps = [{"x": xh[b], "w": w_sb} for b in range(B)]
    res = run_bass_kernel_spmd(nc, in_maps, core_ids=list(range(B)))
    out = np.stack([res.results[b]["out"].reshape(D, H, W) for b in range(B)])
    return out.astype(np.float32)
